# revision 20
# baseline (speedup 1.0000x reference)
"""Trainium2 Bass kernel for DiscriminatorMADClf.

Computation (reference, fp32):
    h    = elu(x @ Wb0 + bb0)                 # [1024, 1024]
    feat = elu(h @ Wb1 + bb1)                 # [1024, 512]
    M    = feat @ T                           # [1024, 128]
    o_b[j,b] = sum_i exp(-|M[i,b]-M[j,b]|) - 1
    mad  = [feat, o_b] @ Wm + bm              # [1024, 17]
    clf  = feat @ Wc + bc                     # [1024, 10]

Sharding: the pairwise o_b term couples the whole batch, so each of the 8
cores runs an identical program that computes the full projection M (all
1024 rows) and the pairwise sums only for output rows j in [0,128).  The
per-core inputs carry the rows ROTATED so that core c's first 128 rows are
the original rows [128c, 128c+128) — the i-sum is permutation invariant, so
no collectives or dynamic addressing are needed.

Device math (verified vs reference):
  * everything flows transposed ([feature, row]) so the contraction dim is
    on partitions; only x needs transposing, done on host.
  * elu(z)+1 == max(z+1, exp(min(z,0))): per tile, DVE computes
    m = min(z+b, 0), ACT computes e = exp(m), DVE computes
    max(z+(b+1), e) via scalar_tensor_tensor.  Working with elu+1 instead
    of elu shifts each GEMM input by +1; the shift is folded into the next
    bias on host (bb1' = bb1 - Wb1.sum(0), bm' = bm - Wm.sum(0),
    bc' = bc - Wc.sum(0)).  The M shift (colsum(T)) cancels inside
    |M_i - M_j|, and the o_b "-1" self-term is also folded into bm'.
  * pairwise inner loop per j: DVE tensor_scalar(subtract, abs_max) gives
    |M - M[:,j]| in one op; ACT activation(Exp, scale=-1, accum_out) gives
    exp(-|d|) summed over i in one op.
  * all tensors the TensorEngine reads are host-packed into ONE DRAM array
    ("wpe") loaded by ONE dma_start: walrus only allows a single sync-wait
    on a Matmult, so every matmul's DMA dependency must be one semaphore.
    The ELU min/max ops are placed on DVE (not ACT) for the same reason:
    PSUM tiles are then only ever read by DVE, so a recycled PSUM slot
    costs a matmul one DVE wait, not DVE+ACT.
"""

import numpy as np

import concourse.bass as bass
import concourse.tile as tile
from concourse import mybir
from concourse.bass_utils import run_bass_kernel_spmd

N, D_IN, H, F, B = 1024, 512, 1024, 512, 128
NM1, NCLS = 17, 10
NCORES = 8
RB = N // NCORES  # 128 output rows per core
FP32 = mybir.dt.float32
F32R = mybir.dt.float32r
BF16 = mybir.dt.bfloat16
AOP = mybir.AluOpType
AF = mybir.ActivationFunctionType

# offsets into the packed weight array (free dim, fp32 elements)
_O_WB0 = 0
_O_XT = _O_WB0 + (D_IN // 128) * H          # 4096
_O_WB1 = _O_XT + (D_IN // 128) * N          # 8192
_O_TM = _O_WB1 + (H // 128) * F             # 12288
_O_WM = _O_TM + (F // 128) * B              # 12800
_O_WC = _O_WM + ((F + B) // 128) * NM1      # 12885
_WPE_F = _O_WC + (F // 128) * NCLS          # 12925
# offsets into the packed bias array (fp32)
_O_BB0 = 0
_O_BB0P1 = _O_BB0 + H // 128                # 8
_O_BB1 = _O_BB0P1 + H // 128                # 16
_O_BB1P1 = _O_BB1 + F // 128                # 20
_O_BMD = _O_BB1P1 + F // 128                # 24
_O_BCD = _O_BMD + 1                         # 25
_BCON_F = _O_BCD + 1                        # 26

_cached = {}


def _legalize_single_wait(nc: bass.Bass) -> None:
    """The walrus build in this container accepts only ONE sync-wait per
    instruction (setupSyncWait raises "Too many sync wait commands" even for
    two engine-sem waits — reproduced on the stock tile_groupnorm kernel).
    Tile freely emits multi-wait instructions, so hoist all but one wait of
    each instruction onto NoOps inserted immediately before it on the same
    engine: the engine blocks on the NoOp's wait first, then the real
    instruction's — semantically identical to an atomic multi-wait."""
    n = 0
    for func in nc.m.functions:
        for block in func.blocks:
            out = []
            for inst in block.instructions:
                si = inst.sync_info
                waits = list(si.on_wait) if si is not None and si.on_wait else []
                if len(waits) > 1:
                    for w in waits[:-1]:
                        nop = mybir.InstNoOp(name=f"I-wsplit-{n}")
                        n += 1
                        nop.engine = inst.engine
                        nop.sync_info = mybir.SyncInfo(on_wait=[w], on_update=[])
                        out.append(nop)
                    inst.sync_info = mybir.SyncInfo(
                        on_wait=[waits[-1]],
                        on_update=list(si.on_update or []),
                    )
                out.append(inst)
            block.instructions = out


def _build_program(legalize: bool = True) -> bass.Bass:
    nc = bass.Bass("TRN2")

    wpe = nc.dram_tensor("wpe", [128, _WPE_F], F32R, kind="ExternalInput")
    bcon = nc.dram_tensor("bcon", [128, _BCON_F], FP32, kind="ExternalInput")

    madT = nc.dram_tensor("madT", [NM1, RB], FP32, kind="ExternalOutput")
    clfT = nc.dram_tensor("clfT", [NCLS, RB], FP32, kind="ExternalOutput")

    KH = H // 128     # 8 h-chunks
    KD = D_IN // 128  # 4 d_in-chunks
    KF = F // 128     # 4 f-chunks
    NRS = 2           # row slabs of 512
    RS = N // NRS

    with tile.TileContext(nc) as tc:
        with (
            tc.tile_pool(name="consts", bufs=1) as consts,
            tc.tile_pool(name="acts", bufs=1) as actsp,
            tc.tile_pool(name="work", bufs=3) as work,
            tc.tile_pool(name="esc", bufs=2) as escp,
            tc.tile_pool(name="zp", bufs=4, space="PSUM") as zp,
            tc.tile_pool(name="headps", bufs=1, space="PSUM") as headps,
        ):
            # ---- loads: interleaved 1MB chunks (wb0_k, xt_k pairs) so
            # GEMM1's k-streamed passes start as soon as chunk 0 lands,
            # then the remaining weights, then biases ----
            wpe_t = consts.tile([128, _WPE_F], F32R)
            for k in range(KD):
                nc.sync.dma_start(out=wpe_t[:, _O_WB0 + k * H:_O_WB0 + (k + 1) * H],
                                  in_=wpe[:, _O_WB0 + k * H:_O_WB0 + (k + 1) * H])
                nc.sync.dma_start(out=wpe_t[:, _O_XT + k * N:_O_XT + (k + 1) * N],
                                  in_=wpe[:, _O_XT + k * N:_O_XT + (k + 1) * N])
            nc.sync.dma_start(out=wpe_t[:, _O_WB1:_WPE_F],
                              in_=wpe[:, _O_WB1:_WPE_F])
            bcon_t = consts.tile([128, _BCON_F], FP32)
            nc.sync.dma_start(out=bcon_t, in_=bcon[:, :])
            wb0_t = wpe_t[:, _O_WB0:_O_XT].rearrange("p (c h) -> p c h", c=KD)
            xt_t = wpe_t[:, _O_XT:_O_WB1].rearrange("p (c r) -> p c r", c=KD)
            wb1_t = wpe_t[:, _O_WB1:_O_TM].rearrange("p (c f) -> p c f", c=KH)
            tm_t = wpe_t[:, _O_TM:_O_WM].rearrange("p (c b) -> p c b", c=KF)
            wm_t = wpe_t[:, _O_WM:_O_WC].rearrange("p (c m) -> p c m", c=KF + 1)
            wc_t = wpe_t[:, _O_WC:_WPE_F].rearrange("p (c m) -> p c m", c=KF)
            bb0_t = bcon_t[:, _O_BB0:_O_BB0P1]
            bb0p1_t = bcon_t[:, _O_BB0P1:_O_BB1]
            bb1_t = bcon_t[:, _O_BB1:_O_BB1P1]
            bb1p1_t = bcon_t[:, _O_BB1P1:_O_BMD]
            bmd_t = bcon_t[0:NM1, _O_BMD:_O_BMD + 1]
            bcd_t = bcon_t[0:NCLS, _O_BCD:_O_BCD + 1]

            # DVE observes the bias-DMA semaphore up front so its first real
            # consumer doesn't need a second wait on one instruction.
            touch = consts.tile([1, 1], FP32)
            nc.vector.tensor_copy(touch, bcon_t[0:1, 0:1])

            # ---- persistent activations ----
            h1p_t = actsp.tile([128, KH, N], F32R)    # (h+1)^T
            featp_t = actsp.tile([128, KF, N], FP32)  # (feat+1)^T
            mt_t = actsp.tile([128, N], FP32)         # M^T (+colsum(T), cancels)
            obt_t = actsp.tile([128, RB], FP32)       # o_b^T (+1, folded into bmd)

            def elu_layer(z, out_ap, bias, bias_p1):
                """out = elu(z + bias) + 1 with z in PSUM (read only by DVE)."""
                m = work.tile([128, RS], FP32, tag="min")
                nc.vector.tensor_scalar(out=m, in0=z, scalar1=bias, scalar2=0.0,
                                        op0=AOP.add, op1=AOP.min)
                e = work.tile([128, RS], FP32, tag="exp")
                nc.scalar.activation(e, m, AF.Exp)
                nc.vector.scalar_tensor_tensor(
                    out=out_ap, in0=z, scalar=bias_p1, in1=e,
                    op0=AOP.add, op1=AOP.max,
                )

            # ---- layer 1: (h+1)^T = elu(x @ Wb0 + bb0)^T + 1 ----
            # k-outer with 4 concurrent PSUM groups: pass k only needs the
            # k-th wb0/xt chunks, so PE starts ~5us in (first 1MB DMA pair)
            # instead of after the whole load.
            for rs in range(NRS):
                for hcg in range(KH // 4):
                    zs = [zp.tile([128, RS], FP32, tag="z", name=f"z_{rs}_{hcg}_{i}")
                          for i in range(4)]
                    for k in range(KD):
                        for i in range(4):
                            hc = hcg * 4 + i
                            nc.tensor.matmul(
                                zs[i],
                                wb0_t[:, k, hc * 128:(hc + 1) * 128],
                                xt_t[:, k, rs * RS:(rs + 1) * RS],
                                start=(k == 0), stop=(k == KD - 1),
                            )
                    for i in range(4):
                        hc = hcg * 4 + i
                        elu_layer(zs[i], h1p_t[:, hc, rs * RS:(rs + 1) * RS],
                                  bb0_t[:, hc:hc + 1], bb0p1_t[:, hc:hc + 1])

            # ---- layer 2: (feat+1)^T = elu(h @ Wb1 + bb1')^T + 1 ----
            for fc in range(KF):
                for rs in range(NRS):
                    z = zp.tile([128, RS], FP32, tag="z")
                    for k in range(KH):
                        nc.tensor.matmul(
                            z,
                            wb1_t[:, k, fc * 128:(fc + 1) * 128],
                            h1p_t[:, k, rs * RS:(rs + 1) * RS],
                            start=(k == 0), stop=(k == KH - 1),
                        )
                    elu_layer(z, featp_t[:, fc, rs * RS:(rs + 1) * RS],
                              bb1_t[:, fc:fc + 1], bb1p1_t[:, fc:fc + 1])

            # ---- M^T = T^T-contraction with feat^T ----
            for rs in range(NRS):
                z = zp.tile([128, RS], FP32, tag="z")
                for k in range(KF):
                    nc.tensor.matmul(
                        z, tm_t[:, k, :].bitcast(FP32),
                        featp_t[:, k, rs * RS:(rs + 1) * RS],
                        start=(k == 0), stop=(k == KF - 1),
                    )
                nc.vector.tensor_copy(mt_t[:, rs * RS:(rs + 1) * RS], z)

            # ---- pairwise: o_b^T[:, j] = sum_i exp(-|M^T - M^T[:, j]|) ----
            # Per j: DVE subtract (tensor_scalar, 2x mode) -> DVE |d| via
            # sign-bit clear (bitwise AND on the uint32 view — tensor_scalar
            # class keeps its fast mode; scalar_tensor_tensor min(-d,d) only
            # runs 1x) -> ACT exp(-|d|) + free-dim accumulate.
            # (GPSIMD was tried for the subtract: ~15us/op and it starves DVE
            # of SBUF ports — avoid.)
            U32 = mybir.dt.uint32
            for j in range(RB):
                d = work.tile([128, N], FP32, tag="diff")
                nc.vector.tensor_scalar(
                    out=d, in0=mt_t, scalar1=mt_t[:, j:j + 1], scalar2=None,
                    op0=AOP.subtract,
                )
                ad = work.tile([128, N], FP32, tag="absd")
                nc.vector.tensor_scalar(
                    out=ad.bitcast(U32), in0=d.bitcast(U32),
                    scalar1=0x7FFFFFFF, scalar2=None,
                    op0=AOP.bitwise_and,
                )
                esc = escp.tile([128, N], FP32, tag="esc")
                nc.scalar.activation(
                    esc, ad, AF.Exp, scale=-1.0, accum_out=obt_t[:, j:j + 1],
                )

            # ---- heads (only this core's rows = first RB columns) ----
            mad_ps = headps.tile([NM1, RB], FP32, tag="mad")
            for q in range(KF):
                nc.tensor.matmul(mad_ps, wm_t[:, q, :].bitcast(FP32),
                                 featp_t[:, q, 0:RB],
                                 start=(q == 0), stop=False)
            nc.tensor.matmul(mad_ps, wm_t[:, KF, :].bitcast(FP32), obt_t,
                             start=False, stop=True)
            mad_sb = work.tile([NM1, RB], FP32, tag="mad_sb")
            nc.vector.tensor_scalar(out=mad_sb, in0=mad_ps, scalar1=bmd_t,
                                    scalar2=None, op0=AOP.add)
            nc.sync.dma_start(out=madT[:, :], in_=mad_sb)

            clf_ps = headps.tile([NCLS, RB], FP32, tag="clf")
            for q in range(KF):
                nc.tensor.matmul(clf_ps, wc_t[:, q, :].bitcast(FP32),
                                 featp_t[:, q, 0:RB],
                                 start=(q == 0), stop=(q == KF - 1))
            clf_sb = work.tile([NCLS, RB], FP32, tag="clf_sb")
            nc.vector.tensor_scalar(out=clf_sb, in0=clf_ps, scalar1=bcd_t,
                                    scalar2=None, op0=AOP.add)
            nc.sync.dma_start(out=clfT[:, :], in_=clf_sb)

    if legalize:
        _legalize_single_wait(nc)
    return nc


def _chunk128(a):
    """[c*128, m] -> [128, c*m] with chunk-major free layout."""
    c = a.shape[0] // 128
    return a.reshape(c, 128, -1).transpose(1, 0, 2).reshape(128, -1)


def _host_inputs(x, Wb0, bb0, Wb1, bb1, T, Wm, bm, Wc, bc):
    """Per-core input maps with host-side folds (cheap numpy)."""
    f32 = np.float32
    x = np.asarray(x, f32)
    Wb0 = np.asarray(Wb0, f32)
    Wb1 = np.asarray(Wb1, f32)
    T = np.asarray(T, f32)
    Wm = np.asarray(Wm, f32)
    Wc = np.asarray(Wc, f32)
    bb0 = np.asarray(bb0, f32)
    bb1_dev = np.asarray(bb1, f32) - Wb1.sum(0)
    bm_dev = np.asarray(bm, f32) - Wm.sum(0)
    bc_dev = np.asarray(bc, f32) - Wc.sum(0)

    bmd_col = np.zeros((128, 1), f32)
    bmd_col[:NM1, 0] = bm_dev
    bcd_col = np.zeros((128, 1), f32)
    bcd_col[:NCLS, 0] = bc_dev
    bcon = np.concatenate([
        bb0.reshape(H // 128, 128).T, (bb0 + 1.0).reshape(H // 128, 128).T,
        bb1_dev.reshape(F // 128, 128).T, (bb1_dev + 1.0).reshape(F // 128, 128).T,
        bmd_col, bcd_col,
    ], axis=1)
    assert bcon.shape == (128, _BCON_F), bcon.shape
    bcon = np.ascontiguousarray(bcon)
    wtail = [_chunk128(Wb1), _chunk128(T), _chunk128(Wm), _chunk128(Wc)]
    wb0_p = _chunk128(Wb0)
    in_maps = []
    for c in range(NCORES):
        xc = np.roll(x, -c * RB, axis=0)
        xt_p = _chunk128(np.ascontiguousarray(xc.T))
        wpe = np.concatenate([wb0_p, xt_p] + wtail, axis=1)
        assert wpe.shape == (128, _WPE_F), wpe.shape
        in_maps.append({"wpe": np.ascontiguousarray(wpe), "bcon": bcon})
    return in_maps


def kernel(x, Wb0, bb0, Wb1, bb1, T, Wm, bm, Wc, bc, _trace=False):
    if "nc" not in _cached:
        _cached["nc"] = _build_program()
    nc = _cached["nc"]

    in_maps = _host_inputs(x, Wb0, bb0, Wb1, bb1, T, Wm, bm, Wc, bc)
    res = run_bass_kernel_spmd(nc, in_maps, core_ids=list(range(NCORES)),
                               trace=_trace)
    _cached["last_result"] = res

    mad = np.empty((N, NM1), np.float32)
    clf = np.empty((N, NCLS), np.float32)
    for c, r in enumerate(res.results):
        mad[c * RB:(c + 1) * RB] = r["madT"].T
        clf[c * RB:(c + 1) * RB] = r["clfT"].T
    return mad, clf


# revision 22
# speedup vs baseline: 1.0172x; 1.0172x over previous
"""Trainium2 Bass kernel for DiscriminatorMADClf.

Computation (reference, fp32):
    h    = elu(x @ Wb0 + bb0)                 # [1024, 1024]
    feat = elu(h @ Wb1 + bb1)                 # [1024, 512]
    M    = feat @ T                           # [1024, 128]
    o_b[j,b] = sum_i exp(-|M[i,b]-M[j,b]|) - 1
    mad  = [feat, o_b] @ Wm + bm              # [1024, 17]
    clf  = feat @ Wc + bc                     # [1024, 10]

Sharding: the pairwise o_b term couples the whole batch, so each of the 8
cores runs an identical program that computes the full projection M (all
1024 rows) and the pairwise sums only for output rows j in [0,128).  The
per-core inputs carry the rows ROTATED so that core c's first 128 rows are
the original rows [128c, 128c+128) — the i-sum is permutation invariant, so
no collectives or dynamic addressing are needed.

Device math (verified vs reference):
  * everything flows transposed ([feature, row]) so the contraction dim is
    on partitions; only x needs transposing, done on host.
  * elu(z)+1 == max(z+1, exp(min(z,0))): per tile, DVE computes
    m = min(z+b, 0), ACT computes e = exp(m), DVE computes
    max(z+(b+1), e) via scalar_tensor_tensor.  Working with elu+1 instead
    of elu shifts each GEMM input by +1; the shift is folded into the next
    bias on host (bb1' = bb1 - Wb1.sum(0), bm' = bm - Wm.sum(0),
    bc' = bc - Wc.sum(0)).  The M shift (colsum(T)) cancels inside
    |M_i - M_j|, and the o_b "-1" self-term is also folded into bm'.
  * pairwise inner loop per j: DVE tensor_scalar(subtract, abs_max) gives
    |M - M[:,j]| in one op; ACT activation(Exp, scale=-1, accum_out) gives
    exp(-|d|) summed over i in one op.
  * all tensors the TensorEngine reads are host-packed into ONE DRAM array
    ("wpe") loaded by ONE dma_start: walrus only allows a single sync-wait
    on a Matmult, so every matmul's DMA dependency must be one semaphore.
    The ELU min/max ops are placed on DVE (not ACT) for the same reason:
    PSUM tiles are then only ever read by DVE, so a recycled PSUM slot
    costs a matmul one DVE wait, not DVE+ACT.
"""

import numpy as np

import concourse.bass as bass
import concourse.tile as tile
from concourse import mybir
from concourse.bass_utils import run_bass_kernel_spmd

N, D_IN, H, F, B = 1024, 512, 1024, 512, 128
NM1, NCLS = 17, 10
NCORES = 8
RB = N // NCORES  # 128 output rows per core
FP32 = mybir.dt.float32
F32R = mybir.dt.float32r
BF16 = mybir.dt.bfloat16
AOP = mybir.AluOpType
AF = mybir.ActivationFunctionType

# offsets into the packed weight array (free dim, fp32 elements)
_O_WB0 = 0
_O_XT = _O_WB0 + (D_IN // 128) * H          # 4096
_O_WB1 = _O_XT + (D_IN // 128) * N          # 8192
_O_TM = _O_WB1 + (H // 128) * F             # 12288
_O_WM = _O_TM + (F // 128) * B              # 12800
_O_WC = _O_WM + ((F + B) // 128) * NM1      # 12885
_WPE_F = _O_WC + (F // 128) * NCLS          # 12925
# offsets into the packed bias array (fp32)
_O_BB0 = 0
_O_BB0P1 = _O_BB0 + H // 128                # 8
_O_BB1 = _O_BB0P1 + H // 128                # 16
_O_BB1P1 = _O_BB1 + F // 128                # 20
_O_BMD = _O_BB1P1 + F // 128                # 24
_O_BCD = _O_BMD + 1                         # 25
_BCON_F = _O_BCD + 1                        # 26

_cached = {}


def _legalize_single_wait(nc: bass.Bass) -> None:
    """The walrus build in this container accepts only ONE sync-wait per
    instruction (setupSyncWait raises "Too many sync wait commands" even for
    two engine-sem waits — reproduced on the stock tile_groupnorm kernel).
    Tile freely emits multi-wait instructions, so hoist all but one wait of
    each instruction onto NoOps inserted immediately before it on the same
    engine: the engine blocks on the NoOp's wait first, then the real
    instruction's — semantically identical to an atomic multi-wait."""
    n = 0
    for func in nc.m.functions:
        for block in func.blocks:
            out = []
            for inst in block.instructions:
                si = inst.sync_info
                waits = list(si.on_wait) if si is not None and si.on_wait else []
                if len(waits) > 1:
                    for w in waits[:-1]:
                        nop = mybir.InstNoOp(name=f"I-wsplit-{n}")
                        n += 1
                        nop.engine = inst.engine
                        nop.sync_info = mybir.SyncInfo(on_wait=[w], on_update=[])
                        out.append(nop)
                    inst.sync_info = mybir.SyncInfo(
                        on_wait=[waits[-1]],
                        on_update=list(si.on_update or []),
                    )
                out.append(inst)
            block.instructions = out


def _build_program(legalize: bool = True) -> bass.Bass:
    nc = bass.Bass("TRN2")

    wpe = nc.dram_tensor("wpe", [128, _WPE_F], F32R, kind="ExternalInput")
    bcon = nc.dram_tensor("bcon", [128, _BCON_F], FP32, kind="ExternalInput")

    madT = nc.dram_tensor("madT", [NM1, RB], FP32, kind="ExternalOutput")
    clfT = nc.dram_tensor("clfT", [NCLS, RB], FP32, kind="ExternalOutput")

    KH = H // 128     # 8 h-chunks
    KD = D_IN // 128  # 4 d_in-chunks
    KF = F // 128     # 4 f-chunks
    NRS = 2           # row slabs of 512
    RS = N // NRS

    with tile.TileContext(nc) as tc:
        with (
            tc.tile_pool(name="consts", bufs=1) as consts,
            tc.tile_pool(name="acts", bufs=1) as actsp,
            tc.tile_pool(name="work", bufs=3) as work,
            tc.tile_pool(name="esc", bufs=2) as escp,
            tc.tile_pool(name="zp", bufs=8, space="PSUM") as zp,
        ):
            # ---- loads: interleaved 1MB chunks (wb0_k, xt_k pairs) so
            # GEMM1's k-streamed passes start as soon as chunk 0 lands,
            # then the remaining weights, then biases ----
            wpe_t = consts.tile([128, _WPE_F], F32R)
            for k in range(KD):
                nc.sync.dma_start(out=wpe_t[:, _O_WB0 + k * H:_O_WB0 + (k + 1) * H],
                                  in_=wpe[:, _O_WB0 + k * H:_O_WB0 + (k + 1) * H])
                nc.sync.dma_start(out=wpe_t[:, _O_XT + k * N:_O_XT + (k + 1) * N],
                                  in_=wpe[:, _O_XT + k * N:_O_XT + (k + 1) * N])
            nc.sync.dma_start(out=wpe_t[:, _O_WB1:_WPE_F],
                              in_=wpe[:, _O_WB1:_WPE_F])
            bcon_t = consts.tile([128, _BCON_F], FP32)
            nc.sync.dma_start(out=bcon_t, in_=bcon[:, :])
            wb0_t = wpe_t[:, _O_WB0:_O_XT].rearrange("p (c h) -> p c h", c=KD)
            xt_t = wpe_t[:, _O_XT:_O_WB1].rearrange("p (c r) -> p c r", c=KD)
            wb1_t = wpe_t[:, _O_WB1:_O_TM].rearrange("p (c f) -> p c f", c=KH)
            tm_t = wpe_t[:, _O_TM:_O_WM].rearrange("p (c b) -> p c b", c=KF)
            wm_t = wpe_t[:, _O_WM:_O_WC].rearrange("p (c m) -> p c m", c=KF + 1)
            wc_t = wpe_t[:, _O_WC:_WPE_F].rearrange("p (c m) -> p c m", c=KF)
            bb0_t = bcon_t[:, _O_BB0:_O_BB0P1]
            bb0p1_t = bcon_t[:, _O_BB0P1:_O_BB1]
            bb1_t = bcon_t[:, _O_BB1:_O_BB1P1]
            bb1p1_t = bcon_t[:, _O_BB1P1:_O_BMD]
            bmd_t = bcon_t[0:NM1, _O_BMD:_O_BMD + 1]
            bcd_t = bcon_t[0:NCLS, _O_BCD:_O_BCD + 1]

            # DVE observes the bias-DMA semaphore up front so its first real
            # consumer doesn't need a second wait on one instruction.
            touch = consts.tile([1, 1], FP32)
            nc.vector.tensor_copy(touch, bcon_t[0:1, 0:1])

            # ---- persistent activations ----
            h1p_t = actsp.tile([128, KH, N], F32R)    # (h+1)^T
            featp_t = actsp.tile([128, KF, N], FP32)  # (feat+1)^T
            mt_t = actsp.tile([128, N], FP32)         # M^T (+colsum(T), cancels)
            obt_t = actsp.tile([128, RB], FP32)       # o_b^T (+1, folded into bmd)

            def elu_layer(z, out_ap, bias, bias_p1):
                """out = elu(z + bias) + 1 with z in PSUM (read only by DVE)."""
                m = work.tile([128, RS], FP32, tag="min")
                nc.vector.tensor_scalar(out=m, in0=z, scalar1=bias, scalar2=0.0,
                                        op0=AOP.add, op1=AOP.min)
                e = work.tile([128, RS], FP32, tag="exp")
                nc.scalar.activation(e, m, AF.Exp)
                nc.vector.scalar_tensor_tensor(
                    out=out_ap, in0=z, scalar=bias_p1, in1=e,
                    op0=AOP.add, op1=AOP.max,
                )

            # ---- layer 1: (h+1)^T = elu(x @ Wb0 + bb0)^T + 1 ----
            # k-outer (pass k only needs the k-th 1MB wb0/xt DMA chunks, so
            # PE starts as soon as the first chunks land) with rs innermost
            # (each lhsT serves both row slabs -> one LDWEIGHTS per chunk).
            # 4 hc x 2 rs = 8 live PSUM groups.
            for hcg in range(KH // 4):
                zs = [zp.tile([128, RS], FP32, tag="z", name=f"z1_{hcg}_{i}")
                      for i in range(8)]
                for k in range(KD):
                    for i in range(4):
                        hc = hcg * 4 + i
                        for rs in range(NRS):
                            nc.tensor.matmul(
                                zs[i * 2 + rs],
                                wb0_t[:, k, hc * 128:(hc + 1) * 128],
                                xt_t[:, k, rs * RS:(rs + 1) * RS],
                                start=(k == 0), stop=(k == KD - 1),
                            )
                for i in range(4):
                    hc = hcg * 4 + i
                    for rs in range(NRS):
                        elu_layer(zs[i * 2 + rs],
                                  h1p_t[:, hc, rs * RS:(rs + 1) * RS],
                                  bb0_t[:, hc:hc + 1], bb0p1_t[:, hc:hc + 1])

            # ---- layer 2: (feat+1)^T = elu(h @ Wb1 + bb1')^T + 1 ----
            zs2 = [zp.tile([128, RS], FP32, tag="z", name=f"z2_{i}")
                   for i in range(8)]
            for k in range(KH):
                for fc in range(KF):
                    for rs in range(NRS):
                        nc.tensor.matmul(
                            zs2[fc * 2 + rs],
                            wb1_t[:, k, fc * 128:(fc + 1) * 128],
                            h1p_t[:, k, rs * RS:(rs + 1) * RS],
                            start=(k == 0), stop=(k == KH - 1),
                        )
            for fc in range(KF):
                for rs in range(NRS):
                    elu_layer(zs2[fc * 2 + rs],
                              featp_t[:, fc, rs * RS:(rs + 1) * RS],
                              bb1_t[:, fc:fc + 1], bb1p1_t[:, fc:fc + 1])

            # ---- M^T = T^T-contraction with feat^T ----
            zms = [zp.tile([128, RS], FP32, tag="z", name=f"zm_{rs}")
                   for rs in range(NRS)]
            for k in range(KF):
                for rs in range(NRS):
                    nc.tensor.matmul(
                        zms[rs], tm_t[:, k, :].bitcast(FP32),
                        featp_t[:, k, rs * RS:(rs + 1) * RS],
                        start=(k == 0), stop=(k == KF - 1),
                    )
            for rs in range(NRS):
                nc.vector.tensor_copy(mt_t[:, rs * RS:(rs + 1) * RS], zms[rs])

            # ---- pairwise: o_b^T[:, j] = sum_i exp(-|M^T - M^T[:, j]|) ----
            # Per j: DVE subtract (tensor_scalar, 2x mode) -> DVE |d| via
            # sign-bit clear (bitwise AND on the uint32 view — tensor_scalar
            # class keeps its fast mode; scalar_tensor_tensor min(-d,d) only
            # runs 1x) -> ACT exp(-|d|) + free-dim accumulate.
            # (GPSIMD was tried for the subtract: ~15us/op and it starves DVE
            # of SBUF ports — avoid.)
            U32 = mybir.dt.uint32
            for j in range(RB):
                d = work.tile([128, N], FP32, tag="diff")
                nc.vector.tensor_scalar(
                    out=d, in0=mt_t, scalar1=mt_t[:, j:j + 1], scalar2=None,
                    op0=AOP.subtract,
                )
                ad = work.tile([128, N], FP32, tag="absd")
                nc.vector.tensor_scalar(
                    out=ad.bitcast(U32), in0=d.bitcast(U32),
                    scalar1=0x7FFFFFFF, scalar2=None,
                    op0=AOP.bitwise_and,
                )
                esc = escp.tile([128, N], FP32, tag="esc")
                nc.scalar.activation(
                    esc, ad, AF.Exp, scale=-1.0, accum_out=obt_t[:, j:j + 1],
                )

            # ---- heads (only this core's rows = first RB columns) ----
            mad_ps = zp.tile([NM1, RB], FP32, tag="z", name="mad_ps")
            for q in range(KF):
                nc.tensor.matmul(mad_ps, wm_t[:, q, :].bitcast(FP32),
                                 featp_t[:, q, 0:RB],
                                 start=(q == 0), stop=False)
            nc.tensor.matmul(mad_ps, wm_t[:, KF, :].bitcast(FP32), obt_t,
                             start=False, stop=True)
            mad_sb = work.tile([NM1, RB], FP32, tag="mad_sb")
            nc.vector.tensor_scalar(out=mad_sb, in0=mad_ps, scalar1=bmd_t,
                                    scalar2=None, op0=AOP.add)
            nc.sync.dma_start(out=madT[:, :], in_=mad_sb)

            clf_ps = zp.tile([NCLS, RB], FP32, tag="z", name="clf_ps")
            for q in range(KF):
                nc.tensor.matmul(clf_ps, wc_t[:, q, :].bitcast(FP32),
                                 featp_t[:, q, 0:RB],
                                 start=(q == 0), stop=(q == KF - 1))
            clf_sb = work.tile([NCLS, RB], FP32, tag="clf_sb")
            nc.vector.tensor_scalar(out=clf_sb, in0=clf_ps, scalar1=bcd_t,
                                    scalar2=None, op0=AOP.add)
            nc.sync.dma_start(out=clfT[:, :], in_=clf_sb)

    if legalize:
        _legalize_single_wait(nc)
    return nc


def _chunk128(a):
    """[c*128, m] -> [128, c*m] with chunk-major free layout."""
    c = a.shape[0] // 128
    return a.reshape(c, 128, -1).transpose(1, 0, 2).reshape(128, -1)


def _host_inputs(x, Wb0, bb0, Wb1, bb1, T, Wm, bm, Wc, bc):
    """Per-core input maps with host-side folds (cheap numpy)."""
    f32 = np.float32
    x = np.asarray(x, f32)
    Wb0 = np.asarray(Wb0, f32)
    Wb1 = np.asarray(Wb1, f32)
    T = np.asarray(T, f32)
    Wm = np.asarray(Wm, f32)
    Wc = np.asarray(Wc, f32)
    bb0 = np.asarray(bb0, f32)
    bb1_dev = np.asarray(bb1, f32) - Wb1.sum(0)
    bm_dev = np.asarray(bm, f32) - Wm.sum(0)
    bc_dev = np.asarray(bc, f32) - Wc.sum(0)

    bmd_col = np.zeros((128, 1), f32)
    bmd_col[:NM1, 0] = bm_dev
    bcd_col = np.zeros((128, 1), f32)
    bcd_col[:NCLS, 0] = bc_dev
    bcon = np.concatenate([
        bb0.reshape(H // 128, 128).T, (bb0 + 1.0).reshape(H // 128, 128).T,
        bb1_dev.reshape(F // 128, 128).T, (bb1_dev + 1.0).reshape(F // 128, 128).T,
        bmd_col, bcd_col,
    ], axis=1)
    assert bcon.shape == (128, _BCON_F), bcon.shape
    bcon = np.ascontiguousarray(bcon)
    wtail = [_chunk128(Wb1), _chunk128(T), _chunk128(Wm), _chunk128(Wc)]
    wb0_p = _chunk128(Wb0)
    in_maps = []
    for c in range(NCORES):
        xc = np.roll(x, -c * RB, axis=0)
        xt_p = _chunk128(np.ascontiguousarray(xc.T))
        wpe = np.concatenate([wb0_p, xt_p] + wtail, axis=1)
        assert wpe.shape == (128, _WPE_F), wpe.shape
        in_maps.append({"wpe": np.ascontiguousarray(wpe), "bcon": bcon})
    return in_maps


def kernel(x, Wb0, bb0, Wb1, bb1, T, Wm, bm, Wc, bc, _trace=False):
    if "nc" not in _cached:
        _cached["nc"] = _build_program()
    nc = _cached["nc"]

    in_maps = _host_inputs(x, Wb0, bb0, Wb1, bb1, T, Wm, bm, Wc, bc)
    res = run_bass_kernel_spmd(nc, in_maps, core_ids=list(range(NCORES)),
                               trace=_trace)
    _cached["last_result"] = res

    mad = np.empty((N, NM1), np.float32)
    clf = np.empty((N, NCLS), np.float32)
    for c, r in enumerate(res.results):
        mad[c * RB:(c + 1) * RB] = r["madT"].T
        clf[c * RB:(c + 1) * RB] = r["clfT"].T
    return mad, clf


# revision 24
# speedup vs baseline: 1.0326x; 1.0151x over previous
"""Trainium2 Bass kernel for DiscriminatorMADClf.

Computation (reference, fp32):
    h    = elu(x @ Wb0 + bb0)                 # [1024, 1024]
    feat = elu(h @ Wb1 + bb1)                 # [1024, 512]
    M    = feat @ T                           # [1024, 128]
    o_b[j,b] = sum_i exp(-|M[i,b]-M[j,b]|) - 1
    mad  = [feat, o_b] @ Wm + bm              # [1024, 17]
    clf  = feat @ Wc + bc                     # [1024, 10]

Sharding: the pairwise o_b term couples the whole batch, so each of the 8
cores runs an identical program that computes the full projection M (all
1024 rows) and the pairwise sums only for output rows j in [0,128).  The
per-core inputs carry the rows ROTATED so that core c's first 128 rows are
the original rows [128c, 128c+128) — the i-sum is permutation invariant, so
no collectives or dynamic addressing are needed.

Device math (verified vs reference):
  * everything flows transposed ([feature, row]) so the contraction dim is
    on partitions; only x needs transposing, done on host.
  * elu(z)+1 == max(z+1, exp(min(z,0))): per tile, DVE computes
    m = min(z+b, 0), ACT computes e = exp(m), DVE computes
    max(z+(b+1), e) via scalar_tensor_tensor.  Working with elu+1 instead
    of elu shifts each GEMM input by +1; the shift is folded into the next
    bias on host (bb1' = bb1 - Wb1.sum(0), bm' = bm - Wm.sum(0),
    bc' = bc - Wc.sum(0)).  The M shift (colsum(T)) cancels inside
    |M_i - M_j|, and the o_b "-1" self-term is also folded into bm'.
  * pairwise inner loop per j (both DVE ops run in a fast perf mode, so
    DVE ~ ACT ~ saturated): DVE tensor_scalar subtract -> DVE |d| via
    sign-bit clear (bitwise AND 0x7FFFFFFF on the uint32 view; the
    min(-d,d) scalar_tensor_tensor form only runs 1x) -> ACT
    activation(Exp, scale=-1, accum_out) = exp(-|d|) summed over i.
  * GEMM1/GEMM2 run as float32r (~1.8 cyc/row vs 4 for fp32; end-to-end
    rel err ~8e-4); the M projection and heads stay fp32.  GEMM1 is
    k-outer over 8 live PSUM groups so PE starts when the first 0.5MB
    wb0/xt DMA chunks land, with rs innermost so each lhsT loads once.
  * walrus here accepts only ONE sync-wait per instruction, so a post-Tile
    pass (_legalize_single_wait) hoists extra waits onto same-engine NoOps.
"""

import numpy as np

import concourse.bass as bass
import concourse.tile as tile
from concourse import mybir
from concourse.bass_utils import run_bass_kernel_spmd

N, D_IN, H, F, B = 1024, 512, 1024, 512, 128
NM1, NCLS = 17, 10
NCORES = 8
RB = N // NCORES  # 128 output rows per core
FP32 = mybir.dt.float32
F32R = mybir.dt.float32r
BF16 = mybir.dt.bfloat16
AOP = mybir.AluOpType
AF = mybir.ActivationFunctionType

# offsets into the packed weight array (free dim, fp32 elements)
_O_WB0 = 0
_O_XT = _O_WB0 + (D_IN // 128) * H          # 4096
_O_WB1 = _O_XT + (D_IN // 128) * N          # 8192
_O_TM = _O_WB1 + (H // 128) * F             # 12288
_O_WM = _O_TM + (F // 128) * B              # 12800
_O_WC = _O_WM + ((F + B) // 128) * NM1      # 12885
_WPE_F = _O_WC + (F // 128) * NCLS          # 12925
# offsets into the packed bias array (fp32)
_O_BB0 = 0
_O_BB0P1 = _O_BB0 + H // 128                # 8
_O_BB1 = _O_BB0P1 + H // 128                # 16
_O_BB1P1 = _O_BB1 + F // 128                # 20
_O_BMD = _O_BB1P1 + F // 128                # 24
_O_BCD = _O_BMD + 1                         # 25
_BCON_F = _O_BCD + 1                        # 26

_cached = {}


def _legalize_single_wait(nc: bass.Bass) -> None:
    """The walrus build in this container accepts only ONE sync-wait per
    instruction (setupSyncWait raises "Too many sync wait commands" even for
    two engine-sem waits — reproduced on the stock tile_groupnorm kernel).
    Tile freely emits multi-wait instructions, so hoist all but one wait of
    each instruction onto NoOps inserted immediately before it on the same
    engine: the engine blocks on the NoOp's wait first, then the real
    instruction's — semantically identical to an atomic multi-wait."""
    n = 0
    for func in nc.m.functions:
        for block in func.blocks:
            out = []
            for inst in block.instructions:
                si = inst.sync_info
                waits = list(si.on_wait) if si is not None and si.on_wait else []
                if len(waits) > 1:
                    for w in waits[:-1]:
                        nop = mybir.InstNoOp(name=f"I-wsplit-{n}")
                        n += 1
                        nop.engine = inst.engine
                        nop.sync_info = mybir.SyncInfo(on_wait=[w], on_update=[])
                        out.append(nop)
                    inst.sync_info = mybir.SyncInfo(
                        on_wait=[waits[-1]],
                        on_update=list(si.on_update or []),
                    )
                out.append(inst)
            block.instructions = out


def _build_program(legalize: bool = True) -> bass.Bass:
    nc = bass.Bass("TRN2")

    wpe = nc.dram_tensor("wpe", [128, _WPE_F], F32R, kind="ExternalInput")
    bcon = nc.dram_tensor("bcon", [128, _BCON_F], FP32, kind="ExternalInput")

    madT = nc.dram_tensor("madT", [NM1, RB], FP32, kind="ExternalOutput")
    clfT = nc.dram_tensor("clfT", [NCLS, RB], FP32, kind="ExternalOutput")

    KH = H // 128     # 8 h-chunks
    KD = D_IN // 128  # 4 d_in-chunks
    KF = F // 128     # 4 f-chunks
    NRS = 2           # row slabs of 512
    RS = N // NRS

    with tile.TileContext(nc) as tc:
        with (
            tc.tile_pool(name="consts", bufs=1) as consts,
            tc.tile_pool(name="acts", bufs=1) as actsp,
            tc.tile_pool(name="work", bufs=3) as work,
            tc.tile_pool(name="esc", bufs=2) as escp,
            tc.tile_pool(name="zp", bufs=8, space="PSUM") as zp,
        ):
            # ---- loads: interleaved 1MB chunks (wb0_k, xt_k pairs) so
            # GEMM1's k-streamed passes start as soon as chunk 0 lands,
            # then the remaining weights, then biases ----
            wpe_t = consts.tile([128, _WPE_F], F32R)
            bcon_t = consts.tile([128, _BCON_F], FP32)
            nc.sync.dma_start(out=bcon_t, in_=bcon[:, :])
            for k in range(KD):
                nc.sync.dma_start(out=wpe_t[:, _O_WB0 + k * H:_O_WB0 + (k + 1) * H],
                                  in_=wpe[:, _O_WB0 + k * H:_O_WB0 + (k + 1) * H])
                nc.sync.dma_start(out=wpe_t[:, _O_XT + k * N:_O_XT + (k + 1) * N],
                                  in_=wpe[:, _O_XT + k * N:_O_XT + (k + 1) * N])
            # wb1 (needed at GEMM2) and the small tail in two pieces; issued
            # last so they contend least with the chunks GEMM1 stalls on.
            half = _O_WB1 + (KH // 2) * F
            nc.sync.dma_start(out=wpe_t[:, _O_WB1:half], in_=wpe[:, _O_WB1:half])
            nc.sync.dma_start(out=wpe_t[:, half:_WPE_F], in_=wpe[:, half:_WPE_F])
            wb0_t = wpe_t[:, _O_WB0:_O_XT].rearrange("p (c h) -> p c h", c=KD)
            xt_t = wpe_t[:, _O_XT:_O_WB1].rearrange("p (c r) -> p c r", c=KD)
            wb1_t = wpe_t[:, _O_WB1:_O_TM].rearrange("p (c f) -> p c f", c=KH)
            tm_t = wpe_t[:, _O_TM:_O_WM].rearrange("p (c b) -> p c b", c=KF)
            wm_t = wpe_t[:, _O_WM:_O_WC].rearrange("p (c m) -> p c m", c=KF + 1)
            wc_t = wpe_t[:, _O_WC:_WPE_F].rearrange("p (c m) -> p c m", c=KF)
            bb0_t = bcon_t[:, _O_BB0:_O_BB0P1]
            bb0p1_t = bcon_t[:, _O_BB0P1:_O_BB1]
            bb1_t = bcon_t[:, _O_BB1:_O_BB1P1]
            bb1p1_t = bcon_t[:, _O_BB1P1:_O_BMD]
            bmd_t = bcon_t[0:NM1, _O_BMD:_O_BMD + 1]
            bcd_t = bcon_t[0:NCLS, _O_BCD:_O_BCD + 1]

            # DVE observes the bias-DMA semaphore up front so its first real
            # consumer doesn't need a second wait on one instruction.
            touch = consts.tile([1, 1], FP32)
            nc.vector.tensor_copy(touch, bcon_t[0:1, 0:1])

            # ---- persistent activations ----
            h1p_t = actsp.tile([128, KH, N], F32R)    # (h+1)^T
            featp_t = actsp.tile([128, KF, N], FP32)  # (feat+1)^T
            mt_t = actsp.tile([128, N], FP32)         # M^T (+colsum(T), cancels)
            obt_t = actsp.tile([128, RB], FP32)       # o_b^T (+1, folded into bmd)

            def elu_layer(z, out_ap, bias, bias_p1):
                """out = elu(z + bias) + 1 with z in PSUM (read only by DVE)."""
                m = work.tile([128, RS], FP32, tag="min")
                nc.vector.tensor_scalar(out=m, in0=z, scalar1=bias, scalar2=0.0,
                                        op0=AOP.add, op1=AOP.min)
                e = work.tile([128, RS], FP32, tag="exp")
                nc.scalar.activation(e, m, AF.Exp)
                nc.vector.scalar_tensor_tensor(
                    out=out_ap, in0=z, scalar=bias_p1, in1=e,
                    op0=AOP.add, op1=AOP.max,
                )

            # ---- layer 1: (h+1)^T = elu(x @ Wb0 + bb0)^T + 1 ----
            # k-outer (pass k only needs the k-th 1MB wb0/xt DMA chunks, so
            # PE starts as soon as the first chunks land) with rs innermost
            # (each lhsT serves both row slabs -> one LDWEIGHTS per chunk).
            # 4 hc x 2 rs = 8 live PSUM groups.
            for hcg in range(KH // 4):
                zs = [zp.tile([128, RS], FP32, tag="z", name=f"z1_{hcg}_{i}")
                      for i in range(8)]
                for k in range(KD):
                    for i in range(4):
                        hc = hcg * 4 + i
                        for rs in range(NRS):
                            nc.tensor.matmul(
                                zs[i * 2 + rs],
                                wb0_t[:, k, hc * 128:(hc + 1) * 128],
                                xt_t[:, k, rs * RS:(rs + 1) * RS],
                                start=(k == 0), stop=(k == KD - 1),
                            )
                for i in range(4):
                    hc = hcg * 4 + i
                    for rs in range(NRS):
                        elu_layer(zs[i * 2 + rs],
                                  h1p_t[:, hc, rs * RS:(rs + 1) * RS],
                                  bb0_t[:, hc:hc + 1], bb0p1_t[:, hc:hc + 1])

            # ---- layer 2: (feat+1)^T = elu(h @ Wb1 + bb1')^T + 1 ----
            zs2 = [zp.tile([128, RS], FP32, tag="z", name=f"z2_{i}")
                   for i in range(8)]
            for k in range(KH):
                for fc in range(KF):
                    for rs in range(NRS):
                        nc.tensor.matmul(
                            zs2[fc * 2 + rs],
                            wb1_t[:, k, fc * 128:(fc + 1) * 128],
                            h1p_t[:, k, rs * RS:(rs + 1) * RS],
                            start=(k == 0), stop=(k == KH - 1),
                        )
            for fc in range(KF):
                for rs in range(NRS):
                    elu_layer(zs2[fc * 2 + rs],
                              featp_t[:, fc, rs * RS:(rs + 1) * RS],
                              bb1_t[:, fc:fc + 1], bb1p1_t[:, fc:fc + 1])

            # ---- M^T = T^T-contraction with feat^T ----
            zms = [zp.tile([128, RS], FP32, tag="z", name=f"zm_{rs}")
                   for rs in range(NRS)]
            for k in range(KF):
                for rs in range(NRS):
                    nc.tensor.matmul(
                        zms[rs], tm_t[:, k, :].bitcast(FP32),
                        featp_t[:, k, rs * RS:(rs + 1) * RS],
                        start=(k == 0), stop=(k == KF - 1),
                    )
            for rs in range(NRS):
                nc.vector.tensor_copy(mt_t[:, rs * RS:(rs + 1) * RS], zms[rs])

            # ---- pairwise: o_b^T[:, j] = sum_i exp(-|M^T - M^T[:, j]|) ----
            # Per j: DVE subtract (tensor_scalar, 2x mode) -> DVE |d| via
            # sign-bit clear (bitwise AND on the uint32 view — tensor_scalar
            # class keeps its fast mode; scalar_tensor_tensor min(-d,d) only
            # runs 1x) -> ACT exp(-|d|) + free-dim accumulate.
            # (GPSIMD was tried for the subtract: ~15us/op and it starves DVE
            # of SBUF ports — avoid.)
            U32 = mybir.dt.uint32
            for j in range(RB):
                d = work.tile([128, N], FP32, tag="diff")
                nc.vector.tensor_scalar(
                    out=d, in0=mt_t, scalar1=mt_t[:, j:j + 1], scalar2=None,
                    op0=AOP.subtract,
                )
                ad = work.tile([128, N], FP32, tag="absd")
                nc.vector.tensor_scalar(
                    out=ad.bitcast(U32), in0=d.bitcast(U32),
                    scalar1=0x7FFFFFFF, scalar2=None,
                    op0=AOP.bitwise_and,
                )
                esc = escp.tile([128, N], FP32, tag="esc")
                nc.scalar.activation(
                    esc, ad, AF.Exp, scale=-1.0, accum_out=obt_t[:, j:j + 1],
                )

            # ---- heads (only this core's rows = first RB columns) ----
            mad_ps = zp.tile([NM1, RB], FP32, tag="z", name="mad_ps")
            for q in range(KF):
                nc.tensor.matmul(mad_ps, wm_t[:, q, :].bitcast(FP32),
                                 featp_t[:, q, 0:RB],
                                 start=(q == 0), stop=False)
            nc.tensor.matmul(mad_ps, wm_t[:, KF, :].bitcast(FP32), obt_t,
                             start=False, stop=True)
            mad_sb = work.tile([NM1, RB], FP32, tag="mad_sb")
            nc.vector.tensor_scalar(out=mad_sb, in0=mad_ps, scalar1=bmd_t,
                                    scalar2=None, op0=AOP.add)
            nc.sync.dma_start(out=madT[:, :], in_=mad_sb)

            clf_ps = zp.tile([NCLS, RB], FP32, tag="z", name="clf_ps")
            for q in range(KF):
                nc.tensor.matmul(clf_ps, wc_t[:, q, :].bitcast(FP32),
                                 featp_t[:, q, 0:RB],
                                 start=(q == 0), stop=(q == KF - 1))
            clf_sb = work.tile([NCLS, RB], FP32, tag="clf_sb")
            nc.vector.tensor_scalar(out=clf_sb, in0=clf_ps, scalar1=bcd_t,
                                    scalar2=None, op0=AOP.add)
            nc.sync.dma_start(out=clfT[:, :], in_=clf_sb)

    if legalize:
        _legalize_single_wait(nc)
    return nc


def _chunk128(a):
    """[c*128, m] -> [128, c*m] with chunk-major free layout."""
    c = a.shape[0] // 128
    return a.reshape(c, 128, -1).transpose(1, 0, 2).reshape(128, -1)


def _host_inputs(x, Wb0, bb0, Wb1, bb1, T, Wm, bm, Wc, bc):
    """Per-core input maps with host-side folds (cheap numpy)."""
    f32 = np.float32
    x = np.asarray(x, f32)
    Wb0 = np.asarray(Wb0, f32)
    Wb1 = np.asarray(Wb1, f32)
    T = np.asarray(T, f32)
    Wm = np.asarray(Wm, f32)
    Wc = np.asarray(Wc, f32)
    bb0 = np.asarray(bb0, f32)
    bb1_dev = np.asarray(bb1, f32) - Wb1.sum(0)
    bm_dev = np.asarray(bm, f32) - Wm.sum(0)
    bc_dev = np.asarray(bc, f32) - Wc.sum(0)

    bmd_col = np.zeros((128, 1), f32)
    bmd_col[:NM1, 0] = bm_dev
    bcd_col = np.zeros((128, 1), f32)
    bcd_col[:NCLS, 0] = bc_dev
    bcon = np.concatenate([
        bb0.reshape(H // 128, 128).T, (bb0 + 1.0).reshape(H // 128, 128).T,
        bb1_dev.reshape(F // 128, 128).T, (bb1_dev + 1.0).reshape(F // 128, 128).T,
        bmd_col, bcd_col,
    ], axis=1)
    assert bcon.shape == (128, _BCON_F), bcon.shape
    bcon = np.ascontiguousarray(bcon)
    wtail = [_chunk128(Wb1), _chunk128(T), _chunk128(Wm), _chunk128(Wc)]
    wb0_p = _chunk128(Wb0)
    in_maps = []
    for c in range(NCORES):
        xc = np.roll(x, -c * RB, axis=0)
        xt_p = _chunk128(np.ascontiguousarray(xc.T))
        wpe = np.concatenate([wb0_p, xt_p] + wtail, axis=1)
        assert wpe.shape == (128, _WPE_F), wpe.shape
        in_maps.append({"wpe": np.ascontiguousarray(wpe), "bcon": bcon})
    return in_maps


def kernel(x, Wb0, bb0, Wb1, bb1, T, Wm, bm, Wc, bc, _trace=False):
    if "nc" not in _cached:
        _cached["nc"] = _build_program()
    nc = _cached["nc"]

    in_maps = _host_inputs(x, Wb0, bb0, Wb1, bb1, T, Wm, bm, Wc, bc)
    res = run_bass_kernel_spmd(nc, in_maps, core_ids=list(range(NCORES)),
                               trace=_trace)
    _cached["last_result"] = res

    mad = np.empty((N, NM1), np.float32)
    clf = np.empty((N, NCLS), np.float32)
    for c, r in enumerate(res.results):
        mad[c * RB:(c + 1) * RB] = r["madT"].T
        clf[c * RB:(c + 1) * RB] = r["clfT"].T
    return mad, clf


# revision 25
# speedup vs baseline: 1.0356x; 1.0030x over previous
"""Trainium2 Bass kernel for DiscriminatorMADClf.

Computation (reference, fp32):
    h    = elu(x @ Wb0 + bb0)                 # [1024, 1024]
    feat = elu(h @ Wb1 + bb1)                 # [1024, 512]
    M    = feat @ T                           # [1024, 128]
    o_b[j,b] = sum_i exp(-|M[i,b]-M[j,b]|) - 1
    mad  = [feat, o_b] @ Wm + bm              # [1024, 17]
    clf  = feat @ Wc + bc                     # [1024, 10]

Sharding: the pairwise o_b term couples the whole batch, so each of the 8
cores runs an identical program that computes the full projection M (all
1024 rows) and the pairwise sums only for output rows j in [0,128).  The
per-core inputs carry the rows ROTATED so that core c's first 128 rows are
the original rows [128c, 128c+128) — the i-sum is permutation invariant, so
no collectives or dynamic addressing are needed.

Device math (verified vs reference):
  * everything flows transposed ([feature, row]) so the contraction dim is
    on partitions; only x needs transposing, done on host.
  * elu(z)+1 == max(z+1, exp(min(z,0))): per tile, DVE computes
    m = min(z+b, 0), ACT computes e = exp(m), DVE computes
    max(z+(b+1), e) via scalar_tensor_tensor.  Working with elu+1 instead
    of elu shifts each GEMM input by +1; the shift is folded into the next
    bias on host (bb1' = bb1 - Wb1.sum(0), bm' = bm - Wm.sum(0),
    bc' = bc - Wc.sum(0)).  The M shift (colsum(T)) cancels inside
    |M_i - M_j|, and the o_b "-1" self-term is also folded into bm'.
  * pairwise inner loop per j (both DVE ops run in a fast perf mode, so
    DVE ~ ACT ~ saturated): DVE tensor_scalar subtract -> DVE |d| via
    sign-bit clear (bitwise AND 0x7FFFFFFF on the uint32 view; the
    min(-d,d) scalar_tensor_tensor form only runs 1x) -> ACT
    activation(Exp, scale=-1, accum_out) = exp(-|d|) summed over i.
  * GEMM1/GEMM2 run as float32r (~1.8 cyc/row vs 4 for fp32; end-to-end
    rel err ~8e-4); the M projection and heads stay fp32.  GEMM1 is
    k-outer over 8 live PSUM groups so PE starts when the first 0.5MB
    wb0/xt DMA chunks land, with rs innermost so each lhsT loads once.
  * walrus here accepts only ONE sync-wait per instruction, so a post-Tile
    pass (_legalize_single_wait) hoists extra waits onto same-engine NoOps.
"""

import numpy as np

import concourse.bass as bass
import concourse.tile as tile
from concourse import mybir
from concourse.bass_utils import run_bass_kernel_spmd

N, D_IN, H, F, B = 1024, 512, 1024, 512, 128
NM1, NCLS = 17, 10
NCORES = 8
RB = N // NCORES  # 128 output rows per core
FP32 = mybir.dt.float32
F32R = mybir.dt.float32r
BF16 = mybir.dt.bfloat16
AOP = mybir.AluOpType
AF = mybir.ActivationFunctionType

# offsets into the packed weight array (free dim, fp32 elements)
_O_WB0 = 0
_O_XT = _O_WB0 + (D_IN // 128) * H          # 4096
_O_WB1 = _O_XT + (D_IN // 128) * N          # 8192
_O_TM = _O_WB1 + (H // 128) * F             # 12288
_O_WM = _O_TM + (F // 128) * B              # 12800
_O_WC = _O_WM + ((F + B) // 128) * NM1      # 12885
_WPE_F = _O_WC + (F // 128) * NCLS          # 12925
# offsets into the packed bias array (fp32)
_O_BB0 = 0
_O_BB0P1 = _O_BB0 + H // 128                # 8
_O_BB1 = _O_BB0P1 + H // 128                # 16
_O_BB1P1 = _O_BB1 + F // 128                # 20
_O_BMD = _O_BB1P1 + F // 128                # 24
_O_BCD = _O_BMD + 1                         # 25
_BCON_F = _O_BCD + 1                        # 26

_cached = {}


def _legalize_single_wait(nc: bass.Bass) -> None:
    """The walrus build in this container accepts only ONE sync-wait per
    instruction (setupSyncWait raises "Too many sync wait commands" even for
    two engine-sem waits — reproduced on the stock tile_groupnorm kernel).
    Tile freely emits multi-wait instructions, so hoist all but one wait of
    each instruction onto NoOps inserted immediately before it on the same
    engine: the engine blocks on the NoOp's wait first, then the real
    instruction's — semantically identical to an atomic multi-wait."""
    n = 0
    for func in nc.m.functions:
        for block in func.blocks:
            out = []
            for inst in block.instructions:
                si = inst.sync_info
                waits = list(si.on_wait) if si is not None and si.on_wait else []
                if len(waits) > 1:
                    for w in waits[:-1]:
                        nop = mybir.InstNoOp(name=f"I-wsplit-{n}")
                        n += 1
                        nop.engine = inst.engine
                        nop.sync_info = mybir.SyncInfo(on_wait=[w], on_update=[])
                        out.append(nop)
                    inst.sync_info = mybir.SyncInfo(
                        on_wait=[waits[-1]],
                        on_update=list(si.on_update or []),
                    )
                out.append(inst)
            block.instructions = out


def _build_program(legalize: bool = True) -> bass.Bass:
    nc = bass.Bass("TRN2")

    wpe = nc.dram_tensor("wpe", [128, _WPE_F], F32R, kind="ExternalInput")
    bcon = nc.dram_tensor("bcon", [128, _BCON_F], FP32, kind="ExternalInput")

    madT = nc.dram_tensor("madT", [NM1, RB], FP32, kind="ExternalOutput")
    clfT = nc.dram_tensor("clfT", [NCLS, RB], FP32, kind="ExternalOutput")

    KH = H // 128     # 8 h-chunks
    KD = D_IN // 128  # 4 d_in-chunks
    KF = F // 128     # 4 f-chunks
    NRS = 2           # row slabs of 512
    RS = N // NRS

    with tile.TileContext(nc) as tc:
        with (
            tc.tile_pool(name="consts", bufs=1) as consts,
            tc.tile_pool(name="acts", bufs=1) as actsp,
            tc.tile_pool(name="work", bufs=4) as work,
            tc.tile_pool(name="esc", bufs=3) as escp,
            tc.tile_pool(name="zp", bufs=8, space="PSUM") as zp,
        ):
            # ---- loads: interleaved 1MB chunks (wb0_k, xt_k pairs) so
            # GEMM1's k-streamed passes start as soon as chunk 0 lands,
            # then the remaining weights, then biases ----
            wpe_t = consts.tile([128, _WPE_F], F32R)
            bcon_t = consts.tile([128, _BCON_F], FP32)
            nc.sync.dma_start(out=bcon_t, in_=bcon[:, :])
            for k in range(KD):
                nc.sync.dma_start(out=wpe_t[:, _O_WB0 + k * H:_O_WB0 + (k + 1) * H],
                                  in_=wpe[:, _O_WB0 + k * H:_O_WB0 + (k + 1) * H])
                nc.sync.dma_start(out=wpe_t[:, _O_XT + k * N:_O_XT + (k + 1) * N],
                                  in_=wpe[:, _O_XT + k * N:_O_XT + (k + 1) * N])
            # wb1 (needed at GEMM2) and the small tail in two pieces; issued
            # last so they contend least with the chunks GEMM1 stalls on.
            half = _O_WB1 + (KH // 2) * F
            nc.sync.dma_start(out=wpe_t[:, _O_WB1:half], in_=wpe[:, _O_WB1:half])
            nc.sync.dma_start(out=wpe_t[:, half:_WPE_F], in_=wpe[:, half:_WPE_F])
            wb0_t = wpe_t[:, _O_WB0:_O_XT].rearrange("p (c h) -> p c h", c=KD)
            xt_t = wpe_t[:, _O_XT:_O_WB1].rearrange("p (c r) -> p c r", c=KD)
            wb1_t = wpe_t[:, _O_WB1:_O_TM].rearrange("p (c f) -> p c f", c=KH)
            tm_t = wpe_t[:, _O_TM:_O_WM].rearrange("p (c b) -> p c b", c=KF)
            wm_t = wpe_t[:, _O_WM:_O_WC].rearrange("p (c m) -> p c m", c=KF + 1)
            wc_t = wpe_t[:, _O_WC:_WPE_F].rearrange("p (c m) -> p c m", c=KF)
            bb0_t = bcon_t[:, _O_BB0:_O_BB0P1]
            bb0p1_t = bcon_t[:, _O_BB0P1:_O_BB1]
            bb1_t = bcon_t[:, _O_BB1:_O_BB1P1]
            bb1p1_t = bcon_t[:, _O_BB1P1:_O_BMD]
            bmd_t = bcon_t[0:NM1, _O_BMD:_O_BMD + 1]
            bcd_t = bcon_t[0:NCLS, _O_BCD:_O_BCD + 1]

            # DVE observes the bias-DMA semaphore up front so its first real
            # consumer doesn't need a second wait on one instruction.
            touch = consts.tile([1, 1], FP32)
            nc.vector.tensor_copy(touch, bcon_t[0:1, 0:1])

            # ---- persistent activations ----
            h1p_t = actsp.tile([128, KH, N], F32R)    # (h+1)^T
            featp_t = actsp.tile([128, KF, N], FP32)  # (feat+1)^T
            mt_t = actsp.tile([128, N], FP32)         # M^T (+colsum(T), cancels)
            obt_t = actsp.tile([128, RB], FP32)       # o_b^T (+1, folded into bmd)

            def elu_layer(z, out_ap, bias, bias_p1):
                """out = elu(z + bias) + 1 with z in PSUM (read only by DVE)."""
                m = work.tile([128, RS], FP32, tag="min")
                nc.vector.tensor_scalar(out=m, in0=z, scalar1=bias, scalar2=0.0,
                                        op0=AOP.add, op1=AOP.min)
                e = work.tile([128, RS], FP32, tag="exp")
                nc.scalar.activation(e, m, AF.Exp)
                nc.vector.scalar_tensor_tensor(
                    out=out_ap, in0=z, scalar=bias_p1, in1=e,
                    op0=AOP.add, op1=AOP.max,
                )

            # ---- layer 1: (h+1)^T = elu(x @ Wb0 + bb0)^T + 1 ----
            # k-outer (pass k only needs the k-th 1MB wb0/xt DMA chunks, so
            # PE starts as soon as the first chunks land) with rs innermost
            # (each lhsT serves both row slabs -> one LDWEIGHTS per chunk).
            # 4 hc x 2 rs = 8 live PSUM groups.
            for hcg in range(KH // 4):
                zs = [zp.tile([128, RS], FP32, tag="z", name=f"z1_{hcg}_{i}")
                      for i in range(8)]
                for k in range(KD):
                    for i in range(4):
                        hc = hcg * 4 + i
                        for rs in range(NRS):
                            nc.tensor.matmul(
                                zs[i * 2 + rs],
                                wb0_t[:, k, hc * 128:(hc + 1) * 128],
                                xt_t[:, k, rs * RS:(rs + 1) * RS],
                                start=(k == 0), stop=(k == KD - 1),
                            )
                for i in range(4):
                    hc = hcg * 4 + i
                    for rs in range(NRS):
                        elu_layer(zs[i * 2 + rs],
                                  h1p_t[:, hc, rs * RS:(rs + 1) * RS],
                                  bb0_t[:, hc:hc + 1], bb0p1_t[:, hc:hc + 1])

            # ---- layer 2: (feat+1)^T = elu(h @ Wb1 + bb1')^T + 1 ----
            zs2 = [zp.tile([128, RS], FP32, tag="z", name=f"z2_{i}")
                   for i in range(8)]
            for k in range(KH):
                for fc in range(KF):
                    for rs in range(NRS):
                        nc.tensor.matmul(
                            zs2[fc * 2 + rs],
                            wb1_t[:, k, fc * 128:(fc + 1) * 128],
                            h1p_t[:, k, rs * RS:(rs + 1) * RS],
                            start=(k == 0), stop=(k == KH - 1),
                        )
            for fc in range(KF):
                for rs in range(NRS):
                    elu_layer(zs2[fc * 2 + rs],
                              featp_t[:, fc, rs * RS:(rs + 1) * RS],
                              bb1_t[:, fc:fc + 1], bb1p1_t[:, fc:fc + 1])

            # ---- M^T = T^T-contraction with feat^T ----
            zms = [zp.tile([128, RS], FP32, tag="z", name=f"zm_{rs}")
                   for rs in range(NRS)]
            for k in range(KF):
                for rs in range(NRS):
                    nc.tensor.matmul(
                        zms[rs], tm_t[:, k, :].bitcast(FP32),
                        featp_t[:, k, rs * RS:(rs + 1) * RS],
                        start=(k == 0), stop=(k == KF - 1),
                    )
            for rs in range(NRS):
                nc.vector.tensor_copy(mt_t[:, rs * RS:(rs + 1) * RS], zms[rs])

            # ---- pairwise: o_b^T[:, j] = sum_i exp(-|M^T - M^T[:, j]|) ----
            # Per j: DVE subtract (tensor_scalar, 2x mode) -> DVE |d| via
            # sign-bit clear (bitwise AND on the uint32 view — tensor_scalar
            # class keeps its fast mode; scalar_tensor_tensor min(-d,d) only
            # runs 1x) -> ACT exp(-|d|) + free-dim accumulate.
            # (GPSIMD was tried for the subtract: ~15us/op and it starves DVE
            # of SBUF ports — avoid.)
            U32 = mybir.dt.uint32
            for j in range(RB):
                d = work.tile([128, N], FP32, tag="diff")
                nc.vector.tensor_scalar(
                    out=d, in0=mt_t, scalar1=mt_t[:, j:j + 1], scalar2=None,
                    op0=AOP.subtract,
                )
                ad = work.tile([128, N], FP32, tag="absd")
                nc.vector.tensor_scalar(
                    out=ad.bitcast(U32), in0=d.bitcast(U32),
                    scalar1=0x7FFFFFFF, scalar2=None,
                    op0=AOP.bitwise_and,
                )
                esc = escp.tile([128, N], FP32, tag="esc")
                nc.scalar.activation(
                    esc, ad, AF.Exp, scale=-1.0, accum_out=obt_t[:, j:j + 1],
                )

            # ---- heads (only this core's rows = first RB columns) ----
            mad_ps = zp.tile([NM1, RB], FP32, tag="z", name="mad_ps")
            for q in range(KF):
                nc.tensor.matmul(mad_ps, wm_t[:, q, :].bitcast(FP32),
                                 featp_t[:, q, 0:RB],
                                 start=(q == 0), stop=False)
            nc.tensor.matmul(mad_ps, wm_t[:, KF, :].bitcast(FP32), obt_t,
                             start=False, stop=True)
            mad_sb = work.tile([NM1, RB], FP32, tag="mad_sb")
            nc.vector.tensor_scalar(out=mad_sb, in0=mad_ps, scalar1=bmd_t,
                                    scalar2=None, op0=AOP.add)
            nc.sync.dma_start(out=madT[:, :], in_=mad_sb)

            clf_ps = zp.tile([NCLS, RB], FP32, tag="z", name="clf_ps")
            for q in range(KF):
                nc.tensor.matmul(clf_ps, wc_t[:, q, :].bitcast(FP32),
                                 featp_t[:, q, 0:RB],
                                 start=(q == 0), stop=(q == KF - 1))
            clf_sb = work.tile([NCLS, RB], FP32, tag="clf_sb")
            nc.vector.tensor_scalar(out=clf_sb, in0=clf_ps, scalar1=bcd_t,
                                    scalar2=None, op0=AOP.add)
            nc.sync.dma_start(out=clfT[:, :], in_=clf_sb)

    if legalize:
        _legalize_single_wait(nc)
    return nc


def _chunk128(a):
    """[c*128, m] -> [128, c*m] with chunk-major free layout."""
    c = a.shape[0] // 128
    return a.reshape(c, 128, -1).transpose(1, 0, 2).reshape(128, -1)


def _host_inputs(x, Wb0, bb0, Wb1, bb1, T, Wm, bm, Wc, bc):
    """Per-core input maps with host-side folds (cheap numpy)."""
    f32 = np.float32
    x = np.asarray(x, f32)
    Wb0 = np.asarray(Wb0, f32)
    Wb1 = np.asarray(Wb1, f32)
    T = np.asarray(T, f32)
    Wm = np.asarray(Wm, f32)
    Wc = np.asarray(Wc, f32)
    bb0 = np.asarray(bb0, f32)
    bb1_dev = np.asarray(bb1, f32) - Wb1.sum(0)
    bm_dev = np.asarray(bm, f32) - Wm.sum(0)
    bc_dev = np.asarray(bc, f32) - Wc.sum(0)

    bmd_col = np.zeros((128, 1), f32)
    bmd_col[:NM1, 0] = bm_dev
    bcd_col = np.zeros((128, 1), f32)
    bcd_col[:NCLS, 0] = bc_dev
    bcon = np.concatenate([
        bb0.reshape(H // 128, 128).T, (bb0 + 1.0).reshape(H // 128, 128).T,
        bb1_dev.reshape(F // 128, 128).T, (bb1_dev + 1.0).reshape(F // 128, 128).T,
        bmd_col, bcd_col,
    ], axis=1)
    assert bcon.shape == (128, _BCON_F), bcon.shape
    bcon = np.ascontiguousarray(bcon)
    wtail = [_chunk128(Wb1), _chunk128(T), _chunk128(Wm), _chunk128(Wc)]
    wb0_p = _chunk128(Wb0)
    in_maps = []
    for c in range(NCORES):
        xc = np.roll(x, -c * RB, axis=0)
        xt_p = _chunk128(np.ascontiguousarray(xc.T))
        wpe = np.concatenate([wb0_p, xt_p] + wtail, axis=1)
        assert wpe.shape == (128, _WPE_F), wpe.shape
        in_maps.append({"wpe": np.ascontiguousarray(wpe), "bcon": bcon})
    return in_maps


def kernel(x, Wb0, bb0, Wb1, bb1, T, Wm, bm, Wc, bc, _trace=False):
    if "nc" not in _cached:
        _cached["nc"] = _build_program()
    nc = _cached["nc"]

    in_maps = _host_inputs(x, Wb0, bb0, Wb1, bb1, T, Wm, bm, Wc, bc)
    res = run_bass_kernel_spmd(nc, in_maps, core_ids=list(range(NCORES)),
                               trace=_trace)
    _cached["last_result"] = res

    mad = np.empty((N, NM1), np.float32)
    clf = np.empty((N, NCLS), np.float32)
    for c, r in enumerate(res.results):
        mad[c * RB:(c + 1) * RB] = r["madT"].T
        clf[c * RB:(c + 1) * RB] = r["clfT"].T
    return mad, clf


# revision 28
# speedup vs baseline: 1.0409x; 1.0050x over previous
"""Trainium2 Bass kernel for DiscriminatorMADClf.

Computation (reference, fp32):
    h    = elu(x @ Wb0 + bb0)                 # [1024, 1024]
    feat = elu(h @ Wb1 + bb1)                 # [1024, 512]
    M    = feat @ T                           # [1024, 128]
    o_b[j,b] = sum_i exp(-|M[i,b]-M[j,b]|) - 1
    mad  = [feat, o_b] @ Wm + bm              # [1024, 17]
    clf  = feat @ Wc + bc                     # [1024, 10]

Sharding: the pairwise o_b term couples the whole batch, so each of the 8
cores runs an identical program that computes the full projection M (all
1024 rows) and the pairwise sums only for output rows j in [0,128).  The
per-core inputs carry the rows ROTATED so that core c's first 128 rows are
the original rows [128c, 128c+128) — the i-sum is permutation invariant, so
no collectives or dynamic addressing are needed.

Device math (verified vs reference):
  * everything flows transposed ([feature, row]) so the contraction dim is
    on partitions; only x needs transposing, done on host.
  * elu(z)+1 == max(z+1, exp(min(z,0))): per tile, DVE computes
    m = min(z+b, 0), ACT computes e = exp(m), DVE computes
    max(z+(b+1), e) via scalar_tensor_tensor.  Working with elu+1 instead
    of elu shifts each GEMM input by +1; the shift is folded into the next
    bias on host (bb1' = bb1 - Wb1.sum(0), bm' = bm - Wm.sum(0),
    bc' = bc - Wc.sum(0)).  The M shift (colsum(T)) cancels inside
    |M_i - M_j|, and the o_b "-1" self-term is also folded into bm'.
  * pairwise inner loop per j (both DVE ops run in a fast perf mode, so
    DVE ~ ACT ~ saturated): DVE tensor_scalar subtract -> DVE |d| via
    sign-bit clear (bitwise AND 0x7FFFFFFF on the uint32 view; the
    min(-d,d) scalar_tensor_tensor form only runs 1x) -> ACT
    activation(Exp, scale=-1, accum_out) = exp(-|d|) summed over i.
  * GEMM1/GEMM2 run as float32r (~1.8 cyc/row vs 4 for fp32; end-to-end
    rel err ~8e-4); the M projection and heads stay fp32.  GEMM1 is
    k-outer over 8 live PSUM groups so PE starts when the first 0.5MB
    wb0/xt DMA chunks land, with rs innermost so each lhsT loads once.
  * walrus here accepts only ONE sync-wait per instruction, so a post-Tile
    pass (_legalize_single_wait) hoists extra waits onto same-engine NoOps.
"""

import numpy as np

import concourse.bass as bass
import concourse.tile as tile
from concourse import mybir
from concourse.bass_utils import run_bass_kernel_spmd

N, D_IN, H, F, B = 1024, 512, 1024, 512, 128
NM1, NCLS = 17, 10
NCORES = 8
RB = N // NCORES  # 128 output rows per core
FP32 = mybir.dt.float32
F32R = mybir.dt.float32r
BF16 = mybir.dt.bfloat16
AOP = mybir.AluOpType
AF = mybir.ActivationFunctionType

# offsets into the packed weight array (free dim, fp32 elements)
_O_WB0 = 0
_O_XT = _O_WB0 + (D_IN // 128) * H          # 4096
_O_WB1 = _O_XT + (D_IN // 128) * N          # 8192
_O_TM = _O_WB1 + (H // 128) * F             # 12288
_O_WM = _O_TM + (F // 128) * B              # 12800
_O_WC = _O_WM + ((F + B) // 128) * NM1      # 12885
_WPE_F = _O_WC + (F // 128) * NCLS          # 12925
# offsets into the packed bias array (fp32)
_O_BB0 = 0
_O_BB0P1 = _O_BB0 + H // 128                # 8
_O_BB1 = _O_BB0P1 + H // 128                # 16
_O_BB1P1 = _O_BB1 + F // 128                # 20
_O_BMD = _O_BB1P1 + F // 128                # 24
_O_BCD = _O_BMD + 1                         # 25
_BCON_F = _O_BCD + 1                        # 26

_cached = {}


def _legalize_single_wait(nc: bass.Bass) -> None:
    """The walrus build in this container accepts only ONE sync-wait per
    instruction (setupSyncWait raises "Too many sync wait commands" even for
    two engine-sem waits — reproduced on the stock tile_groupnorm kernel).
    Tile freely emits multi-wait instructions, so hoist all but one wait of
    each instruction onto NoOps inserted immediately before it on the same
    engine: the engine blocks on the NoOp's wait first, then the real
    instruction's — semantically identical to an atomic multi-wait."""
    n = 0
    for func in nc.m.functions:
        for block in func.blocks:
            out = []
            for inst in block.instructions:
                si = inst.sync_info
                waits = list(si.on_wait) if si is not None and si.on_wait else []
                if len(waits) > 1:
                    for w in waits[:-1]:
                        nop = mybir.InstNoOp(name=f"I-wsplit-{n}")
                        n += 1
                        nop.engine = inst.engine
                        nop.sync_info = mybir.SyncInfo(on_wait=[w], on_update=[])
                        out.append(nop)
                    inst.sync_info = mybir.SyncInfo(
                        on_wait=[waits[-1]],
                        on_update=list(si.on_update or []),
                    )
                out.append(inst)
            block.instructions = out


def _build_program(legalize: bool = True) -> bass.Bass:
    nc = bass.Bass("TRN2")

    wpe = nc.dram_tensor("wpe", [128, _WPE_F], F32R, kind="ExternalInput")
    bcon = nc.dram_tensor("bcon", [128, _BCON_F], FP32, kind="ExternalInput")

    madT = nc.dram_tensor("madT", [NM1, RB], FP32, kind="ExternalOutput")
    clfT = nc.dram_tensor("clfT", [NCLS, RB], FP32, kind="ExternalOutput")

    KH = H // 128     # 8 h-chunks
    KD = D_IN // 128  # 4 d_in-chunks
    KF = F // 128     # 4 f-chunks
    NRS = 2           # row slabs of 512
    RS = N // NRS

    with tile.TileContext(nc) as tc:
        with (
            tc.tile_pool(name="consts", bufs=1) as consts,
            tc.tile_pool(name="acts", bufs=1) as actsp,
            tc.tile_pool(name="work", bufs=5) as work,
            tc.tile_pool(name="esc", bufs=3) as escp,
            tc.tile_pool(name="zp", bufs=8, space="PSUM") as zp,
        ):
            # ---- loads: interleaved 1MB chunks (wb0_k, xt_k pairs) so
            # GEMM1's k-streamed passes start as soon as chunk 0 lands,
            # then the remaining weights, then biases ----
            wpe_t = consts.tile([128, _WPE_F], F32R)
            bcon_t = consts.tile([128, _BCON_F], FP32)
            nc.sync.dma_start(out=bcon_t, in_=bcon[:, :])
            for k in range(KD):
                nc.sync.dma_start(out=wpe_t[:, _O_WB0 + k * H:_O_WB0 + (k + 1) * H],
                                  in_=wpe[:, _O_WB0 + k * H:_O_WB0 + (k + 1) * H])
                nc.sync.dma_start(out=wpe_t[:, _O_XT + k * N:_O_XT + (k + 1) * N],
                                  in_=wpe[:, _O_XT + k * N:_O_XT + (k + 1) * N])
            # wb1 (needed at GEMM2) and the small tail in two pieces; issued
            # last so they contend least with the chunks GEMM1 stalls on.
            half = _O_WB1 + (KH // 2) * F
            nc.sync.dma_start(out=wpe_t[:, _O_WB1:half], in_=wpe[:, _O_WB1:half])
            nc.sync.dma_start(out=wpe_t[:, half:_WPE_F], in_=wpe[:, half:_WPE_F])
            wb0_t = wpe_t[:, _O_WB0:_O_XT].rearrange("p (c h) -> p c h", c=KD)
            xt_t = wpe_t[:, _O_XT:_O_WB1].rearrange("p (c r) -> p c r", c=KD)
            wb1_t = wpe_t[:, _O_WB1:_O_TM].rearrange("p (c f) -> p c f", c=KH)
            tm_t = wpe_t[:, _O_TM:_O_WM].rearrange("p (c b) -> p c b", c=KF)
            wm_t = wpe_t[:, _O_WM:_O_WC].rearrange("p (c m) -> p c m", c=KF + 1)
            wc_t = wpe_t[:, _O_WC:_WPE_F].rearrange("p (c m) -> p c m", c=KF)
            bb0_t = bcon_t[:, _O_BB0:_O_BB0P1]
            bb0p1_t = bcon_t[:, _O_BB0P1:_O_BB1]
            bb1_t = bcon_t[:, _O_BB1:_O_BB1P1]
            bb1p1_t = bcon_t[:, _O_BB1P1:_O_BMD]
            bmd_t = bcon_t[0:NM1, _O_BMD:_O_BMD + 1]
            bcd_t = bcon_t[0:NCLS, _O_BCD:_O_BCD + 1]

            # DVE observes the bias-DMA semaphore up front so its first real
            # consumer doesn't need a second wait on one instruction.
            touch = consts.tile([1, 1], FP32)
            nc.vector.tensor_copy(touch, bcon_t[0:1, 0:1])

            # ---- persistent activations ----
            h1p_t = actsp.tile([128, KH, N], F32R)    # (h+1)^T
            featp_t = actsp.tile([128, KF, N], FP32)  # (feat+1)^T
            mt_t = actsp.tile([128, N], FP32)         # M^T (+colsum(T), cancels)
            obt_t = actsp.tile([128, RB], FP32)       # o_b^T (+1, folded into bmd)

            def elu_layer(z, out_ap, bias, bias_p1):
                """out = elu(z + bias) + 1 with z in PSUM (read only by DVE)."""
                m = work.tile([128, RS], FP32, tag="min")
                nc.vector.tensor_scalar(out=m, in0=z, scalar1=bias, scalar2=0.0,
                                        op0=AOP.add, op1=AOP.min)
                e = work.tile([128, RS], FP32, tag="exp")
                nc.scalar.activation(e, m, AF.Exp)
                nc.vector.scalar_tensor_tensor(
                    out=out_ap, in0=z, scalar=bias_p1, in1=e,
                    op0=AOP.add, op1=AOP.max,
                )

            # ---- layer 1: (h+1)^T = elu(x @ Wb0 + bb0)^T + 1 ----
            # k-outer (pass k only needs the k-th 1MB wb0/xt DMA chunks, so
            # PE starts as soon as the first chunks land) with rs innermost
            # (each lhsT serves both row slabs -> one LDWEIGHTS per chunk).
            # 4 hc x 2 rs = 8 live PSUM groups.
            for hcg in range(KH // 4):
                zs = [zp.tile([128, RS], FP32, tag="z", name=f"z1_{hcg}_{i}")
                      for i in range(8)]
                for k in range(KD):
                    for i in range(4):
                        hc = hcg * 4 + i
                        for rs in range(NRS):
                            nc.tensor.matmul(
                                zs[i * 2 + rs],
                                wb0_t[:, k, hc * 128:(hc + 1) * 128],
                                xt_t[:, k, rs * RS:(rs + 1) * RS],
                                start=(k == 0), stop=(k == KD - 1),
                            )
                for i in range(4):
                    hc = hcg * 4 + i
                    for rs in range(NRS):
                        elu_layer(zs[i * 2 + rs],
                                  h1p_t[:, hc, rs * RS:(rs + 1) * RS],
                                  bb0_t[:, hc:hc + 1], bb0p1_t[:, hc:hc + 1])

            # ---- layer 2: (feat+1)^T = elu(h @ Wb1 + bb1')^T + 1 ----
            zs2 = [zp.tile([128, RS], FP32, tag="z", name=f"z2_{i}")
                   for i in range(8)]
            for k in range(KH):
                for fc in range(KF):
                    for rs in range(NRS):
                        nc.tensor.matmul(
                            zs2[fc * 2 + rs],
                            wb1_t[:, k, fc * 128:(fc + 1) * 128],
                            h1p_t[:, k, rs * RS:(rs + 1) * RS],
                            start=(k == 0), stop=(k == KH - 1),
                        )
            for fc in range(KF):
                for rs in range(NRS):
                    elu_layer(zs2[fc * 2 + rs],
                              featp_t[:, fc, rs * RS:(rs + 1) * RS],
                              bb1_t[:, fc:fc + 1], bb1p1_t[:, fc:fc + 1])

            # ---- M^T = T^T-contraction with feat^T ----
            zms = [zp.tile([128, RS], FP32, tag="z", name=f"zm_{rs}")
                   for rs in range(NRS)]
            for k in range(KF):
                for rs in range(NRS):
                    nc.tensor.matmul(
                        zms[rs], tm_t[:, k, :].bitcast(FP32),
                        featp_t[:, k, rs * RS:(rs + 1) * RS],
                        start=(k == 0), stop=(k == KF - 1),
                    )
            for rs in range(NRS):
                nc.vector.tensor_copy(mt_t[:, rs * RS:(rs + 1) * RS], zms[rs])

            # ---- pairwise: o_b^T[:, j] = sum_i exp(-|M^T - M^T[:, j]|) ----
            # Per j: DVE subtract (tensor_scalar, 2x mode) -> DVE |d| via
            # sign-bit clear (bitwise AND on the uint32 view — tensor_scalar
            # class keeps its fast mode; scalar_tensor_tensor min(-d,d) only
            # runs 1x) -> ACT exp(-|d|) + free-dim accumulate.
            # (GPSIMD was tried for the subtract: ~15us/op and it starves DVE
            # of SBUF ports — avoid.)
            U32 = mybir.dt.uint32
            for j in range(RB):
                d = work.tile([128, N], FP32, tag="diff")
                nc.vector.tensor_scalar(
                    out=d, in0=mt_t, scalar1=mt_t[:, j:j + 1], scalar2=None,
                    op0=AOP.subtract,
                )
                ad = work.tile([128, N], FP32, tag="absd")
                nc.vector.tensor_scalar(
                    out=ad.bitcast(U32), in0=d.bitcast(U32),
                    scalar1=0x7FFFFFFF, scalar2=None,
                    op0=AOP.bitwise_and,
                )
                esc = escp.tile([128, N], FP32, tag="esc")
                nc.scalar.activation(
                    esc, ad, AF.Exp, scale=-1.0, accum_out=obt_t[:, j:j + 1],
                )

            # ---- heads (only this core's rows = first RB columns) ----
            mad_ps = zp.tile([NM1, RB], FP32, tag="z", name="mad_ps")
            for q in range(KF):
                nc.tensor.matmul(mad_ps, wm_t[:, q, :].bitcast(FP32),
                                 featp_t[:, q, 0:RB],
                                 start=(q == 0), stop=False)
            nc.tensor.matmul(mad_ps, wm_t[:, KF, :].bitcast(FP32), obt_t,
                             start=False, stop=True)
            mad_sb = work.tile([NM1, RB], FP32, tag="mad_sb")
            nc.vector.tensor_scalar(out=mad_sb, in0=mad_ps, scalar1=bmd_t,
                                    scalar2=None, op0=AOP.add)
            nc.sync.dma_start(out=madT[:, :], in_=mad_sb)

            clf_ps = zp.tile([NCLS, RB], FP32, tag="z", name="clf_ps")
            for q in range(KF):
                nc.tensor.matmul(clf_ps, wc_t[:, q, :].bitcast(FP32),
                                 featp_t[:, q, 0:RB],
                                 start=(q == 0), stop=(q == KF - 1))
            clf_sb = work.tile([NCLS, RB], FP32, tag="clf_sb")
            nc.vector.tensor_scalar(out=clf_sb, in0=clf_ps, scalar1=bcd_t,
                                    scalar2=None, op0=AOP.add)
            nc.sync.dma_start(out=clfT[:, :], in_=clf_sb)

    if legalize:
        _legalize_single_wait(nc)
    return nc


def _chunk128(a):
    """[c*128, m] -> [128, c*m] with chunk-major free layout."""
    c = a.shape[0] // 128
    return a.reshape(c, 128, -1).transpose(1, 0, 2).reshape(128, -1)


def _host_inputs(x, Wb0, bb0, Wb1, bb1, T, Wm, bm, Wc, bc):
    """Per-core input maps with host-side folds (cheap numpy)."""
    f32 = np.float32
    x = np.asarray(x, f32)
    Wb0 = np.asarray(Wb0, f32)
    Wb1 = np.asarray(Wb1, f32)
    T = np.asarray(T, f32)
    Wm = np.asarray(Wm, f32)
    Wc = np.asarray(Wc, f32)
    bb0 = np.asarray(bb0, f32)
    bb1_dev = np.asarray(bb1, f32) - Wb1.sum(0)
    bm_dev = np.asarray(bm, f32) - Wm.sum(0)
    bc_dev = np.asarray(bc, f32) - Wc.sum(0)

    bmd_col = np.zeros((128, 1), f32)
    bmd_col[:NM1, 0] = bm_dev
    bcd_col = np.zeros((128, 1), f32)
    bcd_col[:NCLS, 0] = bc_dev
    bcon = np.concatenate([
        bb0.reshape(H // 128, 128).T, (bb0 + 1.0).reshape(H // 128, 128).T,
        bb1_dev.reshape(F // 128, 128).T, (bb1_dev + 1.0).reshape(F // 128, 128).T,
        bmd_col, bcd_col,
    ], axis=1)
    assert bcon.shape == (128, _BCON_F), bcon.shape
    bcon = np.ascontiguousarray(bcon)
    wtail = [_chunk128(Wb1), _chunk128(T), _chunk128(Wm), _chunk128(Wc)]
    wb0_p = _chunk128(Wb0)
    in_maps = []
    for c in range(NCORES):
        xc = np.roll(x, -c * RB, axis=0)
        xt_p = _chunk128(np.ascontiguousarray(xc.T))
        wpe = np.concatenate([wb0_p, xt_p] + wtail, axis=1)
        assert wpe.shape == (128, _WPE_F), wpe.shape
        in_maps.append({"wpe": np.ascontiguousarray(wpe), "bcon": bcon})
    return in_maps


def kernel(x, Wb0, bb0, Wb1, bb1, T, Wm, bm, Wc, bc, _trace=False):
    if "nc" not in _cached:
        _cached["nc"] = _build_program()
    nc = _cached["nc"]

    in_maps = _host_inputs(x, Wb0, bb0, Wb1, bb1, T, Wm, bm, Wc, bc)
    res = run_bass_kernel_spmd(nc, in_maps, core_ids=list(range(NCORES)),
                               trace=_trace)
    _cached["last_result"] = res

    mad = np.empty((N, NM1), np.float32)
    clf = np.empty((N, NCLS), np.float32)
    for c, r in enumerate(res.results):
        mad[c * RB:(c + 1) * RB] = r["madT"].T
        clf[c * RB:(c + 1) * RB] = r["clfT"].T
    return mad, clf
